# revision 1
# baseline (speedup 1.0000x reference)
"""Trainium2 Bass kernel for nn_Attention_76089640616322.

Bahdanau-style attention:
  B, S, HE, DOUT = 32, 4096, 512, 512  (HD = 1024)
  energy = tanh(concat([context, broadcast(output)], -1) @ W1.T)   [B,S,HE]
  attn   = softmax(energy @ W2.T, axis=S)                           [B,1,S]
  mix    = attn @ context                                           [B,1,HE]
  out    = tanh(concat([mix, output], -1) @ Wout.T + bout)          [B,1,HE]

Sharding: pure data parallel, batch dim across 8 cores (4 batches/core),
weights replicated.

Structure (per core, context batch kept SBUF-resident in bf16):
- The broadcast `output` columns of the concat make W1 @ concat(...) =
  W1[:, :HE] @ context + (W1[:, HE:] @ output_b); the second term is a
  per-batch constant fused as a per-partition bias into the tanh (ACT).
- context is cast fp32->bf16 during the HBM load (SWDGE cast DMA) with a
  per-partition-contiguous rearrange (s = k*512 + 4p + ss) so each DMA
  descriptor covers an 8 KiB run.
- ctx is PE-transposed in bf16 (1 cycle/row) into paired psum tiles with a
  single DVE/ACT copyback per pair; energyT = W1cT.T @ ctxT runs as bf16
  matmuls with fp32 PSUM accumulation (N=512 streams).
- The W2 logit dot and the softmax-weighted mix contraction both use the
  "stationary flip": the large tile (tanhT / ctx chunk) rides the
  weight-load stream and the 1-wide vector is the moving operand, so their
  outputs land directly in column form and cost no 512-cycle PE streams.
- Softmax is computed unnormalized (|logit| <= ||W2||_1 ~ 8, exp safe in
  fp32; exp on ACT with a free per-partition accum for the denominator);
  the normalization is folded in after the mix contraction.
- Batch tails are deferred one batch so their serial chain overlaps the
  next batch's block pipeline.
"""

from contextlib import ExitStack

import numpy as np

import concourse.bass as bass
import concourse.tile as tile
from concourse import bacc, mybir
from concourse._compat import with_exitstack
from concourse.masks import make_identity

B, S, HE, DOUT = 32, 4096, 512, 512
HD = HE + DOUT
NCORES = 8
BC = B // NCORES  # batches per core

F32 = mybir.dt.float32
BF16 = mybir.dt.bfloat16
AF = mybir.ActivationFunctionType

NSBLK = 8       # s-blocks per batch (512 s each)
SBLK = S // NSBLK   # 512
NSS = SBLK // 128   # 4 subtiles of 128 s per block
NEC = HE // 128     # 4 e-chunks
NDC = HE // 128     # 4 d-chunks for the context half of W1
SCHUNKS = S // 128  # 32 s-chunks of 128 per batch


@with_exitstack
def attention_kernel(ctx: ExitStack, tc: tile.TileContext, out_ap, ins):
    nc = tc.nc

    ctx_ap = ins["context"]    # [BC, S, HE]
    outp_ap = ins["output"]    # [BC, 1, DOUT]
    w1_ap = ins["W1"]          # [HE, HD]
    w2_ap = ins["W2"]          # [1, HE]
    wout_ap = ins["Wout"]      # [HE, HD]
    bout_ap = ins["bout"]      # [HE]

    const = ctx.enter_context(tc.tile_pool(name="const", bufs=1))
    ctx_pool = ctx.enter_context(tc.tile_pool(name="ctx", bufs=18))
    ctxT_pool = ctx.enter_context(tc.tile_pool(name="ctxT", bufs=8))
    tanh_pool = ctx.enter_context(tc.tile_pool(name="tanh", bufs=12))
    small = ctx.enter_context(tc.tile_pool(name="small", bufs=2))

    psum_tp = ctx.enter_context(tc.tile_pool(name="ptp", bufs=2, space="PSUM"))
    psum_en = ctx.enter_context(tc.tile_pool(name="pen", bufs=2, space="PSUM"))
    psum_pcol = ctx.enter_context(tc.tile_pool(name="ppcol", bufs=2, space="PSUM"))
    psum_misc = ctx.enter_context(tc.tile_pool(name="pmisc", bufs=2, space="PSUM"))

    # ---- constants ----
    id128f = const.tile([128, 128], F32)
    make_identity(nc, id128f)
    id128b = const.tile([128, 128], BF16)
    nc.vector.tensor_copy(id128b, id128f)
    ones1f = const.tile([1, 1], F32)
    nc.vector.memset(ones1f, 1.0)
    ones128 = const.tile([128, 1], F32)
    nc.vector.memset(ones128, 1.0)
    ones_row = const.tile([1, 128], F32)
    nc.vector.memset(ones_row, 1.0)

    # ---- load weights ----
    w1_t = w1_ap.rearrange("(c p) d -> c p d", p=128)     # [4,128,1024]
    wout_t = wout_ap.rearrange("(c p) d -> c p d", p=128)
    w1sb = []
    woutsb = []
    for c in range(NEC):
        t1 = ctx_pool.tile([128, HD], F32, tag="ctx")
        nc.sync.dma_start(out=t1, in_=w1_t[c])
        w1sb.append(t1)

    def emit_wout_load():
        for c in range(NEC):
            t2 = ctx_pool.tile([128, HD], F32, tag="ctx")
            nc.sync.dma_start(out=t2, in_=wout_t[c])
            woutsb.append(t2)

    w2sb = const.tile([1, HE], F32)
    nc.sync.dma_start(out=w2sb, in_=w2_ap)
    boutsb = const.tile([1, HE], F32)
    nc.sync.dma_start(out=boutsb, in_=bout_ap.rearrange("(a d) -> a d", a=1))
    outp_rows = []
    for b in range(BC):
        t = const.tile([1, DOUT], F32, tag=f"outp_row{b}")
        nc.sync.dma_start(out=t, in_=outp_ap[b])
        outp_rows.append(t)

    # ---- transpose W1 -> W1T (8 tiles [d=128, e=512]) ----
    w1T = []
    for dc in range(HD // 128):
        ps = psum_tp.tile([128, HE], F32, tag="tp")
        for ec in range(NEC):
            nc.tensor.transpose(
                ps[:, ec * 128:(ec + 1) * 128],
                w1sb[ec][:, dc * 128:(dc + 1) * 128],
                id128f,
            )
        dst = const.tile([128, HE], F32, tag=f"w1T{dc}")
        nc.vector.tensor_copy(dst, ps)
        w1T.append(dst)

    woutT = []

    def emit_wout_setup():
        for dc in range(HD // 128):
            ps = psum_tp.tile([128, HE], F32, tag="tp")
            for ec in range(NEC):
                nc.tensor.transpose(
                    ps[:, ec * 128:(ec + 1) * 128],
                    woutsb[ec][:, dc * 128:(dc + 1) * 128],
                    id128f,
                )
            dst = const.tile([128, HE], F32, tag=f"woutT{dc}")
            nc.vector.tensor_copy(dst, ps)
            woutT.append(dst)

    # bf16 stationary for the energy matmul: W1cT = W1T[:4]
    w1cTb = []
    for dc in range(NDC):
        t = const.tile([128, HE], BF16, tag=f"w1cTb{dc}")
        nc.vector.tensor_copy(t, w1T[dc])
        w1cTb.append(t)

    # ---- columnize W2 (bf16), bout, output ----
    ps = psum_misc.tile([128, NEC], F32, tag="misc")
    for ec in range(NEC):
        nc.tensor.transpose(
            ps[:, ec:ec + 1], w2sb[:, ec * 128:(ec + 1) * 128], ones1f
        )
    w2colb = const.tile([128, NEC], BF16)
    nc.vector.tensor_copy(w2colb, ps)

    ps = psum_misc.tile([128, NEC], F32, tag="misc")
    for ec in range(NEC):
        nc.tensor.transpose(
            ps[:, ec:ec + 1], boutsb[:, ec * 128:(ec + 1) * 128], ones1f
        )
    boutcol = const.tile([128, NEC], F32)
    nc.vector.tensor_copy(boutcol, ps)

    # output_b columns: outpcol[:, b*4+dc] = output[b, dc*128 + p]
    ps = psum_misc.tile([128, BC * 4], F32, tag="misc")
    for b in range(BC):
        for dc in range(4):
            nc.tensor.transpose(
                ps[:, b * 4 + dc: b * 4 + dc + 1],
                outp_rows[b][:, dc * 128:(dc + 1) * 128],
                ones1f,
            )
    outpcol = const.tile([128, BC * 4], F32)
    nc.vector.tensor_copy(outpcol, ps)

    # ---- per-batch tanh offsets: off[b] = W1[:, HE:] @ output_b ----
    ps = psum_misc.tile([128, BC * NEC], F32, tag="misc")
    for b in range(BC):
        for ec in range(NEC):
            for dco in range(4):
                nc.tensor.matmul(
                    ps[:, b * NEC + ec: b * NEC + ec + 1],
                    lhsT=w1T[4 + dco][:, ec * 128:(ec + 1) * 128],
                    rhs=outpcol[:, b * 4 + dco: b * 4 + dco + 1],
                    start=(dco == 0),
                    stop=(dco == 3),
                )
    offsb = const.tile([128, BC * NEC], F32)
    nc.vector.tensor_copy(offsb, ps)

    # ---- main loop over batches (tails deferred one batch for overlap) ----
    def emit_blocks(b):
        ctx_b = ctx_ap[b].rearrange("(k p ss) d -> k p ss d", ss=NSS, p=128)
        ctx_tiles = []
        pcol = psum_pcol.tile([128, SCHUNKS], F32)
        lg_pending = [None]

        def logit_mms(kk, tanh_k):
            for ss in range(NSS):
                j = kk * NSS + ss
                for ec in range(NEC):
                    nc.tensor.matmul(
                        pcol[:, j:j + 1],
                        lhsT=tanh_k[ec][:, ss * 128:(ss + 1) * 128],
                        rhs=w2colb[:, ec:ec + 1],
                        start=(ec == 0),
                        stop=(ec == NEC - 1),
                    )

        def load_and_transpose(k):
            # load one s-block [128, 4, 512], casting fp32 -> bf16 in the DMA
            ct = ctx_pool.tile([128, NSS, HE], BF16, tag="ctx")
            nc.gpsimd.dma_start(out=ct, in_=ctx_b[k])

            # PE-transpose to ctxT [d=128, s=512] per d-chunk (bf16, 1 cyc/row)
            # two d-chunks share one psum tile/one DVE copyback to cut DVE SEQ
            ctxT = []
            for dp in range(NDC // 2):
                pt = psum_tp.tile([128, 2, SBLK], BF16, tag="tp")
                for half in range(2):
                    dc = dp * 2 + half
                    for ss in range(NSS):
                        nc.tensor.transpose(
                            pt[:, half, ss * 128:(ss + 1) * 128],
                            ct[:, ss, dc * 128:(dc + 1) * 128],
                            id128b,
                        )
                st = ctxT_pool.tile([128, 2, SBLK], BF16, tag="ctxT")
                nc.vector.tensor_copy(st, pt)
                ctxT.append(st)
            return ct, ctxT

        # transpose stage runs one block ahead so the energy matmuls never
        # wait on the DVE copyback of their own block
        cur = load_and_transpose(0)
        for k in range(NSBLK):
            ct, ctxT = cur
            ctx_tiles.append(ct)
            if k + 1 < NSBLK:
                cur = load_and_transpose(k + 1)

            # energyT[e_chunk, s_blk] = sum_dc W1cT[dc,ec].T @ ctxT[dc]
            tanh_tiles = []
            for ec in range(NEC):
                pe = psum_en.tile([128, SBLK], F32, tag="en")
                for dc in range(NDC):
                    nc.tensor.matmul(
                        pe,
                        lhsT=w1cTb[dc][:, ec * 128:(ec + 1) * 128],
                        rhs=ctxT[dc // 2][:, dc % 2, :],
                        start=(dc == 0),
                        stop=(dc == NDC - 1),
                    )
                th = tanh_pool.tile([128, SBLK], BF16, tag="tanh")
                nc.scalar.activation(
                    th, pe, AF.Tanh, bias=offsb[:, b * NEC + ec: b * NEC + ec + 1]
                )
                tanh_tiles.append(th)

            # logit matvecs for the PREVIOUS block, so the PE's static order
            # never waits on a tanh that ACT has only just been issued
            if lg_pending[0] is not None:
                logit_mms(*lg_pending[0])
            lg_pending[0] = (k, tanh_tiles)
        logit_mms(*lg_pending[0])
        return ctx_tiles, pcol

    def emit_tail(b, ctx_tiles, pcol):
        # exp (unnormalized softmax, bf16 weights) + per-partition fp32 sums
        pexp = small.tile([128, SCHUNKS], BF16, tag="pexp")
        rowsum = small.tile([128, 1], F32, tag="rowsum")
        nc.scalar.activation(pexp, pcol, AF.Exp, accum_out=rowsum)

        pd = psum_misc.tile([1, 1], F32, tag="misc")
        nc.tensor.matmul(pd, lhsT=rowsum, rhs=ones128)
        inv = small.tile([1, 1], F32, tag="inv")
        nc.vector.reciprocal(inv, pd)
        pinvb = psum_misc.tile([128, 1], F32, tag="misc")
        nc.tensor.matmul(pinvb, lhsT=ones_row, rhs=inv)
        invb = small.tile([128, 1], F32, tag="invb")
        nc.vector.tensor_copy(invb, pinvb)

        # mix columns directly: ctx chunks as stationary (LDW stream),
        # exp-weight column as the 1-wide moving operand; accumulate over j
        pmcol = psum_misc.tile([128, 4], F32, tag="misc")
        for dc in range(4):
            for j in range(SCHUNKS):
                nc.tensor.matmul(
                    pmcol[:, dc:dc + 1],
                    lhsT=ctx_tiles[j // NSS][:, j % NSS, dc * 128:(dc + 1) * 128],
                    rhs=pexp[:, j:j + 1],
                    start=(j == 0),
                    stop=(j == SCHUNKS - 1),
                )
        mc = small.tile([128, 4], F32, tag="mc_sb")
        nc.vector.tensor_scalar_mul(mc, pmcol, invb)

        # final: out_col[ec] = sum_dc WoutT[dc,ec].T @ comb_col[dc]
        pfo = psum_misc.tile([128, NEC], F32, tag="misc")
        for ec in range(NEC):
            for dc in range(8):
                rhs = (
                    mc[:, dc:dc + 1]
                    if dc < 4
                    else outpcol[:, b * 4 + (dc - 4): b * 4 + (dc - 4) + 1]
                )
                nc.tensor.matmul(
                    pfo[:, ec:ec + 1],
                    lhsT=woutT[dc][:, ec * 128:(ec + 1) * 128],
                    rhs=rhs,
                    start=(dc == 0),
                    stop=(dc == 7),
                )
        fo = small.tile([128, NEC], F32, tag="fo_sb")
        for ec in range(NEC):
            nc.scalar.activation(
                fo[:, ec:ec + 1], pfo[:, ec:ec + 1], AF.Tanh,
                bias=boutcol[:, ec:ec + 1],
            )

        # back to a row [1, 512] and out
        por = psum_misc.tile([1, HE], F32, tag="misc")
        for ec in range(NEC):
            nc.tensor.transpose(
                por[:, ec * 128:(ec + 1) * 128], fo[:, ec:ec + 1], id128f
            )
        orow = small.tile([1, HE], F32, tag="orow")
        nc.vector.tensor_copy(orow, por)
        nc.sync.dma_start(out=out_ap[b], in_=orow)

    pending = None
    for b in range(BC):
        state = emit_blocks(b)
        if b == 0:
            emit_wout_load()
            emit_wout_setup()
        if pending is not None:
            emit_tail(pending[0], *pending[1])
        pending = (b, state)
    emit_tail(pending[0], *pending[1])


INPUT_SPECS = {
    "output": ((BC, 1, DOUT), F32),
    "context": ((BC, S, HE), F32),
    "W1": ((HE, HD), F32),
    "W2": ((1, HE), F32),
    "Wout": ((HE, HD), F32),
    "bout": ((HE,), F32),
}

_CACHE = {}


def build_nc():
    if "nc" in _CACHE:
        return _CACHE["nc"]
    nc = bacc.Bacc("TRN2", target_bir_lowering=False, debug=False,
                   num_devices=NCORES)
    ins = {
        name: nc.dram_tensor(name, list(shape), dt, kind="ExternalInput").ap()
        for name, (shape, dt) in INPUT_SPECS.items()
    }
    out = nc.dram_tensor("out", [BC, 1, HE], F32, kind="ExternalOutput").ap()
    with tile.TileContext(nc) as tc:
        attention_kernel(tc, out, ins)
    nc.compile()
    _CACHE["nc"] = nc
    return nc


def make_in_maps(output, context, W1, W2, Wout, bout):
    maps = []
    for i in range(NCORES):
        sl = slice(i * BC, (i + 1) * BC)
        maps.append({
            "output": np.ascontiguousarray(output[sl], dtype=np.float32),
            "context": np.ascontiguousarray(context[sl], dtype=np.float32),
            "W1": np.ascontiguousarray(W1, dtype=np.float32),
            "W2": np.ascontiguousarray(W2, dtype=np.float32),
            "Wout": np.ascontiguousarray(Wout, dtype=np.float32),
            "bout": np.ascontiguousarray(bout, dtype=np.float32),
        })
    return maps


def run(inputs, trace=False):
    from concourse.bass_utils import run_bass_kernel_spmd

    nc = build_nc()
    in_maps = make_in_maps(**inputs)
    res = run_bass_kernel_spmd(nc, in_maps, list(range(NCORES)), trace=trace)
    out = np.concatenate([res.results[i]["out"] for i in range(NCORES)], axis=0)
    return out, res


def kernel(output, context, W1, W2, Wout, bout):
    out, _ = run(dict(output=output, context=context, W1=W1, W2=W2,
                      Wout=Wout, bout=bout))
    return out

